# revision 4
# baseline (speedup 1.0000x reference)
import sys

if "/opt/trn_rl_repo" not in sys.path:
    sys.path.insert(0, "/opt/trn_rl_repo")

import numpy as np

B, S, V, D = 256, 512, 100, 64
NCORES = 8
R = B // NCORES  # rows per core
VP = V + 1  # gather K: vocab rows + ones/bias row

PB_BOUNDS = [0, 1024, 3072, 7168]
while PB_BOUNDS[-1] < R * 2 * S:
    PB_BOUNDS.append(min(PB_BOUNDS[-1] + 8192, R * 2 * S))
NPBCH = len(PB_BOUNDS) - 1
PBW = max(b - a for a, b in zip(PB_BOUNDS, PB_BOUNDS[1:]))

# const tile layout (bf16 [128, 356])
C_W1R0 = 0
C_W1R1 = 64
C_B1 = 128
C_W2 = 192
C_ID = 256
CW = 356

_CACHE = {}
LAST_RESULT = None


def _register_mlp1():
    # fused hpre = in0*s0 + in1*s1 (one DVE pass instead of two stt);
    # falls back to the stt path if registration fails
    try:
        from concourse import dve_ops
        from concourse.dve_spec import Spec, Src0, Src1, C0, C1, lower
        from concourse.dve_ops import has_src1
        from concourse.dve_uop import DveOpSpec
        from concourse.dve_table_gen import dve_ver_for

        for o in dve_ops.OPS:
            if o.name == "NCE_MLP1":
                return o
        spec = Spec(
            body=Src0 * C0 + Src1 * C1,
            reference=lambda in0, in1, c0, c1, c2: (
                in0.astype(np.float32) * c0 + in1.astype(np.float32) * c1
            ),
        )
        opcode = dve_ops._CUSTOM_DVE_ROW_BASE + len(dve_ops.OPS)
        shas = {}
        for ver in ("v3", "v4"):
            s = DveOpSpec(
                name="NCE_MLP1", opcode=opcode,
                uops=lower(spec, ver=ver), rd1_en=has_src1(spec),
            )
            shas[ver] = s.sha(ver)
        op = dve_ops.DveOp("NCE_MLP1", spec, subdim=False, uops_sha=shas)
        dve_ops.OPS.append(op)
        dve_ops._SUB_OPCODE_FOR_NAME["NCE_MLP1"] = opcode
        dve_ops.CUSTOM_DVE_SPECS["NCE_MLP1"] = spec
        ver = dve_ver_for("TRN2")
        op.compile(ver)  # sha self-check; raises if anything is off
        return op
    except Exception:
        return None


def _emit(ctx, nc, tc, pb, cst, b1c, b2row, out):
    from concourse import bass

    mlp1 = _register_mlp1()

    mybir = bass.mybir
    f32 = mybir.dt.float32
    bf16 = mybir.dt.bfloat16
    alu = mybir.AluOpType
    act = mybir.ActivationFunctionType

    consts_p = ctx.enter_context(tc.tile_pool(name="cst", bufs=1))
    oh_p = ctx.enter_context(tc.tile_pool(name="oh", bufs=1))
    hsd_p = ctx.enter_context(tc.tile_pool(name="hsd", bufs=4))
    mlp_p = ctx.enter_context(tc.tile_pool(name="mlp", bufs=3))
    hT_p = ctx.enter_context(tc.tile_pool(name="hT", bufs=2))
    tab_p = ctx.enter_context(tc.tile_pool(name="tab", bufs=1))
    gout_p = ctx.enter_context(tc.tile_pool(name="gout", bufs=2))
    ps_t = ctx.enter_context(tc.tile_pool(name="ps_t", bufs=2, space="PSUM"))
    ps_m = ctx.enter_context(tc.tile_pool(name="ps_m", bufs=2, space="PSUM"))
    ps_g = ctx.enter_context(tc.tile_pool(name="ps_g", bufs=1, space="PSUM"))

    # resident prebroadcast ids: pb_sb[v, r*1024 + side*512 + s] = ids - v
    pb_sb = consts_p.tile([V, R * 2 * S], bf16)
    for i, (a, b) in enumerate(zip(PB_BOUNDS, PB_BOUNDS[1:])):
        nc.sync.dma_start(
            out=pb_sb[:, a:b], in_=pb[i * V : (i + 1) * V, 0 : b - a]
        )
    cstt = consts_p.tile([128, CW], bf16)
    nc.scalar.dma_start(out=cstt, in_=cst)
    b1cs = consts_p.tile([D, 1], f32)
    nc.scalar.dma_start(out=b1cs, in_=b1c)

    w1r0b = cstt[0:V, C_W1R0 : C_W1R0 + D]
    w1r1b = cstt[0:V, C_W1R1 : C_W1R1 + D]
    b1b = cstt[0:V, C_B1 : C_B1 + D]
    w2b = cstt[0:D, C_W2 : C_W2 + D]
    identb = cstt[0:V, C_ID : C_ID + V]

    oh_tiles = [oh_p.tile([VP, 2 * S], bf16, name=f"oh{i}") for i in range(6)]
    for t in oh_tiles:
        # base partition must be in {0,32,64,96}; rows 96:100 are
        # overwritten by every compare, row 100 stays 1
        nc.gpsimd.memset(t[96:VP, :], 1.0)
    tab_tiles = [tab_p.tile([VP, D], bf16, name=f"tab{i}") for i in range(2)]
    for t in tab_tiles:
        # rows 96:100 are overwritten by every tab copy, row 100 = b2
        nc.scalar.dma_start(out=t[96:VP, :], in_=b2row)

    psg_tiles = [ps_g.tile([128, 2 * S], f32, name=f"psg{i}") for i in range(2)]
    gout = None

    for r in range(R):
        oh = oh_tiles[r % 6]
        hsd = hsd_p.tile([128, 2], f32)
        base = r * 2 * S
        nc.vector.tensor_scalar(
            out=oh[0:V, 0:S], in0=pb_sb[:, base : base + S],
            scalar1=0.0, scalar2=None, op0=alu.is_equal, op1=alu.add,
            accum_out=hsd[0:V, 0:1],
        )
        nc.vector.tensor_scalar(
            out=oh[0:V, S : 2 * S], in0=pb_sb[:, base + S : base + 2 * S],
            scalar1=0.0, scalar2=None, op0=alu.is_equal, op1=alu.add,
            accum_out=hsd[0:V, 1:2],
        )
        # padding id 0 contributes encode(0, 0)
        nc.gpsimd.memset(hsd[0:1, 0:2], 0.0)

        hpre = mlp_p.tile([V, D], bf16)
        if mlp1 is not None:
            # b1 is applied as the Act bias after the transpose
            nc.vector._custom_dve(
                mlp1, out=hpre, in0=w1r0b, in1=w1r1b,
                s0=hsd[0:V, 0:1], s1=hsd[0:V, 1:2],
            )
        else:
            tmp = mlp_p.tile([V, D], bf16)
            nc.vector.scalar_tensor_tensor(
                out=tmp, in0=w1r0b, scalar=hsd[0:V, 0:1], in1=b1b,
                op0=alu.mult, op1=alu.add,
            )
            nc.vector.scalar_tensor_tensor(
                out=hpre, in0=w1r1b, scalar=hsd[0:V, 1:2], in1=tmp,
                op0=alu.mult, op1=alu.add,
            )
        pst = ps_t.tile([D, V], bf16)
        nc.tensor.transpose(pst, hpre, identb)
        hTr = hT_p.tile([D, V], bf16)
        nc.scalar.activation(
            out=hTr, in_=pst, func=act.Relu,
            bias=b1cs if mlp1 is not None else 0.0, scale=1.0,
        )
        pstab = ps_m.tile([V, D], f32)
        nc.tensor.matmul(out=pstab, lhsT=hTr, rhs=w2b)
        tab = tab_tiles[r % 2]
        nc.scalar.activation(
            out=tab[0:V, :], in_=pstab, func=act.Identity, scale=1.0
        )

        g = r % 2
        psg = psg_tiles[(r // 2) % 2]
        win = slice(g * S, (g + 1) * S)
        nc.tensor.matmul(
            out=psg[0:D, win], lhsT=tab, rhs=oh[:, 0:S], tile_position=(0, 0)
        )
        nc.tensor.matmul(
            out=psg[D : 2 * D, win], lhsT=tab, rhs=oh[:, S : 2 * S],
            tile_position=(0, D),
        )
        if g == 1:
            q = (r // 2) % 2
            if q == 0:
                gout = gout_p.tile([128, 4 * S], bf16)
            nc.scalar.activation(
                out=gout[:, q * 2 * S : (q + 1) * 2 * S], in_=psg,
                func=act.Identity, scale=1.0,
            )
            # last batch ships per 2 rows to shorten the kernel tail
            if r == R - 3:
                nc.gpsimd.dma_start(
                    out=out[:, (r - 1) * S : (r + 1) * S],
                    in_=gout[:, 0 : 2 * S],
                )
            elif r == R - 1:
                nc.gpsimd.dma_start(
                    out=out[:, (r - 1) * S : (r + 1) * S],
                    in_=gout[:, 2 * S : 4 * S],
                )
            elif q == 1:
                nc.gpsimd.dma_start(
                    out=out[:, (r - 3) * S : (r + 1) * S], in_=gout
                )


def _build_module():
    from contextlib import ExitStack

    from concourse import bacc, bass, tile

    mybir = bass.mybir
    nc = bacc.Bacc(
        "TRN2", target_bir_lowering=False, debug=False, num_devices=NCORES
    )
    pb = nc.dram_tensor(
        "pb", [NPBCH * V, PBW], mybir.dt.bfloat16, kind="ExternalInput"
    ).ap()
    cst = nc.dram_tensor(
        "cst", [128, CW], mybir.dt.bfloat16, kind="ExternalInput"
    ).ap()
    b1c = nc.dram_tensor(
        "b1c", [D, 1], mybir.dt.float32, kind="ExternalInput"
    ).ap()
    b2row = nc.dram_tensor(
        "b2row", [5, D], mybir.dt.bfloat16, kind="ExternalInput"
    ).ap()
    out = nc.dram_tensor(
        "out", [128, R * S], mybir.dt.bfloat16, kind="ExternalOutput"
    ).ap()

    with tile.TileContext(nc) as tc:
        with ExitStack() as ctx:
            _emit(ctx, nc, tc, pb, cst, b1c, b2row, out)
    nc.finalize()
    return nc


def get_module():
    if "nc" not in _CACHE:
        _CACHE["nc"] = _build_module()
    return _CACHE["nc"]


def _build_consts(W1, b1, W2):
    import ml_dtypes

    c = np.zeros((128, CW), np.float32)
    c[:, C_W1R0 : C_W1R0 + D] = W1[0]
    c[:, C_W1R1 : C_W1R1 + D] = W1[1]
    c[:, C_B1 : C_B1 + D] = b1
    c[0:D, C_W2 : C_W2 + D] = W2
    c[0:V, C_ID : C_ID + V] = np.eye(V, dtype=np.float32)
    return c.astype(ml_dtypes.bfloat16)


def _build_pb(src, dst):
    # logical pb[v, r*1024 + side*512 + s] = ids[r, s] - v (bf16 exact),
    # stored chunk-major: chunk i at rows [i*V:(i+1)*V], HBM-contiguous
    import ml_dtypes

    ids = np.stack([src, dst], axis=1).astype(np.float32)  # [R, 2, S]
    v = np.arange(V, dtype=np.float32)
    pbf = (ids.reshape(1, R * 2 * S) - v[:, None]).astype(ml_dtypes.bfloat16)
    out = np.zeros((NPBCH * V, PBW), ml_dtypes.bfloat16)
    for i, (a, b) in enumerate(zip(PB_BOUNDS, PB_BOUNDS[1:])):
        out[i * V : (i + 1) * V, 0 : b - a] = pbf[:, a:b]
    return out


def kernel(**inputs):
    global LAST_RESULT
    import ml_dtypes

    from concourse import bass_utils

    src = np.asarray(inputs["src_neighbor_ids"])
    dst = np.asarray(inputs["dst_neighbor_ids"])
    W1 = np.asarray(inputs["W1"], np.float32)
    b1 = np.asarray(inputs["b1"], np.float32)
    W2 = np.asarray(inputs["W2"], np.float32)
    b2 = np.asarray(inputs["b2"], np.float32)

    bf16 = ml_dtypes.bfloat16
    consts = _build_consts(W1, b1, W2)
    b2r = np.tile(b2.reshape(1, D), (5, 1)).astype(bf16)

    in_maps = []
    for c in range(NCORES):
        sl = slice(c * R, (c + 1) * R)
        in_maps.append(
            {
                "pb": _build_pb(src[sl], dst[sl]),
                "cst": consts,
                "b1c": b1.reshape(D, 1),
                "b2row": b2r,
            }
        )

    nc = get_module()
    import os

    trace = bool(int(os.environ.get("KERNEL_TRACE", "0")))
    res = bass_utils.run_bass_kernel_spmd(
        nc, in_maps, core_ids=list(range(NCORES)), trace=trace
    )
    LAST_RESULT = res

    src_feat = np.empty((B, S, D), np.float32)
    dst_feat = np.empty((B, S, D), np.float32)
    for c in range(NCORES):
        o = res.results[c]["out"].astype(np.float32).reshape(128, R, S)
        sl = slice(c * R, (c + 1) * R)
        src_feat[sl] = o[0:D].transpose(1, 2, 0)
        dst_feat[sl] = o[D : 2 * D].transpose(1, 2, 0)
    return src_feat, dst_feat


# revision 5
# speedup vs baseline: 1.0957x; 1.0957x over previous
import sys

if "/opt/trn_rl_repo" not in sys.path:
    sys.path.insert(0, "/opt/trn_rl_repo")

import numpy as np

B, S, V, D = 256, 512, 100, 64
NCORES = 8
R = B // NCORES  # rows per core
VP = V + 1  # gather K: vocab rows + ones/bias row

PB_BOUNDS = [0, 512, 1536, 3584, 7680]
while PB_BOUNDS[-1] < R * 2 * S:
    PB_BOUNDS.append(min(PB_BOUNDS[-1] + 8192, R * 2 * S))
NPBCH = len(PB_BOUNDS) - 1
PBW = max(b - a for a, b in zip(PB_BOUNDS, PB_BOUNDS[1:]))

# const tile layout (bf16 [128, 356])
C_W1R0 = 0
C_W1R1 = 64
C_B1 = 128
C_W2 = 192
C_ID = 256
CW = 356

_CACHE = {}
LAST_RESULT = None


def _register_mlp1():
    # fused hpre = in0*s0 + in1*s1 (one DVE pass instead of two stt);
    # falls back to the stt path if registration fails
    try:
        from concourse import dve_ops
        from concourse.dve_spec import Spec, Src0, Src1, C0, C1, lower
        from concourse.dve_ops import has_src1
        from concourse.dve_uop import DveOpSpec
        from concourse.dve_table_gen import dve_ver_for

        for o in dve_ops.OPS:
            if o.name == "NCE_MLP1":
                return o
        spec = Spec(
            body=Src0 * C0 + Src1 * C1,
            reference=lambda in0, in1, c0, c1, c2: (
                in0.astype(np.float32) * c0 + in1.astype(np.float32) * c1
            ),
        )
        opcode = dve_ops._CUSTOM_DVE_ROW_BASE + len(dve_ops.OPS)
        shas = {}
        for ver in ("v3", "v4"):
            s = DveOpSpec(
                name="NCE_MLP1", opcode=opcode,
                uops=lower(spec, ver=ver), rd1_en=has_src1(spec),
            )
            shas[ver] = s.sha(ver)
        op = dve_ops.DveOp("NCE_MLP1", spec, subdim=False, uops_sha=shas)
        dve_ops.OPS.append(op)
        dve_ops._SUB_OPCODE_FOR_NAME["NCE_MLP1"] = opcode
        dve_ops.CUSTOM_DVE_SPECS["NCE_MLP1"] = spec
        ver = dve_ver_for("TRN2")
        op.compile(ver)  # sha self-check; raises if anything is off
        return op
    except Exception:
        return None


def _emit(ctx, nc, tc, pb, cst, b1c, b2row, out):
    from concourse import bass

    mlp1 = _register_mlp1()

    mybir = bass.mybir
    f32 = mybir.dt.float32
    bf16 = mybir.dt.bfloat16
    alu = mybir.AluOpType
    act = mybir.ActivationFunctionType

    consts_p = ctx.enter_context(tc.tile_pool(name="cst", bufs=1))
    oh_p = ctx.enter_context(tc.tile_pool(name="oh", bufs=1))
    hsd_p = ctx.enter_context(tc.tile_pool(name="hsd", bufs=4))
    mlp_p = ctx.enter_context(tc.tile_pool(name="mlp", bufs=3))
    hT_p = ctx.enter_context(tc.tile_pool(name="hT", bufs=2))
    tab_p = ctx.enter_context(tc.tile_pool(name="tab", bufs=1))
    gout_p = ctx.enter_context(tc.tile_pool(name="gout", bufs=2))
    ps_t = ctx.enter_context(tc.tile_pool(name="ps_t", bufs=2, space="PSUM"))
    ps_m = ctx.enter_context(tc.tile_pool(name="ps_m", bufs=2, space="PSUM"))
    ps_g = ctx.enter_context(tc.tile_pool(name="ps_g", bufs=1, space="PSUM"))

    # resident prebroadcast ids: pb_sb[v, r*1024 + side*512 + s] = ids - v
    pb_sb = consts_p.tile([V, R * 2 * S], bf16)
    # consts first on the Act ring (tiny, Act is idle during startup),
    # then the TAIL half of pb rides the same ring concurrently with the
    # head half on the SP ring; all issued before any Act compute queues
    cstt = consts_p.tile([128, CW], bf16)
    nc.scalar.dma_start(out=cstt, in_=cst)
    b1cs = consts_p.tile([D, 1], f32)
    nc.scalar.dma_start(out=b1cs, in_=b1c)
    tab_tiles = [tab_p.tile([VP, D], bf16, name=f"tab{i}") for i in range(2)]
    for t in tab_tiles:
        # rows 96:100 are overwritten by every tab copy, row 100 = b2
        nc.scalar.dma_start(out=t[96:VP, :], in_=b2row)
    for i, (a, b) in enumerate(zip(PB_BOUNDS, PB_BOUNDS[1:])):
        nc.sync.dma_start(
            out=pb_sb[:, a:b], in_=pb[i * V : (i + 1) * V, 0 : b - a]
        )

    w1r0b = cstt[0:V, C_W1R0 : C_W1R0 + D]
    w1r1b = cstt[0:V, C_W1R1 : C_W1R1 + D]
    b1b = cstt[0:V, C_B1 : C_B1 + D]
    w2b = cstt[0:D, C_W2 : C_W2 + D]
    identb = cstt[0:V, C_ID : C_ID + V]

    oh_tiles = [oh_p.tile([VP, 2 * S], bf16, name=f"oh{i}") for i in range(6)]
    for t in oh_tiles:
        # base partition must be in {0,32,64,96}; rows 96:100 are
        # overwritten by every compare, row 100 stays 1
        nc.gpsimd.memset(t[96:VP, :], 1.0)
    psg_tiles = [ps_g.tile([128, 2 * S], f32, name=f"psg{i}") for i in range(2)]
    gout = None

    for r in range(R):
        oh = oh_tiles[r % 6]
        hsd = hsd_p.tile([128, 2], f32)
        base = r * 2 * S
        nc.vector.tensor_scalar(
            out=oh[0:V, 0:S], in0=pb_sb[:, base : base + S],
            scalar1=0.0, scalar2=None, op0=alu.is_equal, op1=alu.add,
            accum_out=hsd[0:V, 0:1],
        )
        nc.vector.tensor_scalar(
            out=oh[0:V, S : 2 * S], in0=pb_sb[:, base + S : base + 2 * S],
            scalar1=0.0, scalar2=None, op0=alu.is_equal, op1=alu.add,
            accum_out=hsd[0:V, 1:2],
        )
        hpre = mlp_p.tile([V, D], bf16)
        if mlp1 is not None:
            # b1 is applied as the Act bias after the transpose
            nc.vector._custom_dve(
                mlp1, out=hpre, in0=w1r0b, in1=w1r1b,
                s0=hsd[0:V, 0:1], s1=hsd[0:V, 1:2],
            )
        else:
            tmp = mlp_p.tile([V, D], bf16)
            nc.vector.scalar_tensor_tensor(
                out=tmp, in0=w1r0b, scalar=hsd[0:V, 0:1], in1=b1b,
                op0=alu.mult, op1=alu.add,
            )
            nc.vector.scalar_tensor_tensor(
                out=hpre, in0=w1r1b, scalar=hsd[0:V, 1:2], in1=tmp,
                op0=alu.mult, op1=alu.add,
            )
        pst = ps_t.tile([D, V], bf16)
        nc.tensor.transpose(pst, hpre, identb)
        hTr = hT_p.tile([D, V], bf16)
        nc.scalar.activation(
            out=hTr, in_=pst, func=act.Relu,
            bias=b1cs if mlp1 is not None else 0.0, scale=1.0,
        )
        pstab = ps_m.tile([V, D], f32)
        nc.tensor.matmul(out=pstab, lhsT=hTr, rhs=w2b)
        tab = tab_tiles[r % 2]
        nc.scalar.activation(
            out=tab[0:V, :], in_=pstab, func=act.Identity, scale=1.0
        )

        g = r % 2
        psg = psg_tiles[(r // 2) % 2]
        win = slice(g * S, (g + 1) * S)
        nc.tensor.matmul(
            out=psg[0:D, win], lhsT=tab, rhs=oh[:, 0:S], tile_position=(0, 0)
        )
        nc.tensor.matmul(
            out=psg[D : 2 * D, win], lhsT=tab, rhs=oh[:, S : 2 * S],
            tile_position=(0, D),
        )
        if g == 1:
            q = (r // 2) % 2
            if q == 0:
                gout = gout_p.tile([128, 4 * S], bf16)
            nc.scalar.activation(
                out=gout[:, q * 2 * S : (q + 1) * 2 * S], in_=psg,
                func=act.Identity, scale=1.0,
            )
            # last batch ships per 2 rows to shorten the kernel tail
            if r == R - 3:
                nc.gpsimd.dma_start(
                    out=out[:, (r - 1) * S : (r + 1) * S],
                    in_=gout[:, 0 : 2 * S],
                )
            elif r == R - 1:
                nc.gpsimd.dma_start(
                    out=out[:, (r - 1) * S : (r + 1) * S],
                    in_=gout[:, 2 * S : 4 * S],
                )
            elif q == 1:
                nc.gpsimd.dma_start(
                    out=out[:, (r - 3) * S : (r + 1) * S], in_=gout
                )


def _build_module():
    from contextlib import ExitStack

    from concourse import bacc, bass, tile

    mybir = bass.mybir
    nc = bacc.Bacc(
        "TRN2", target_bir_lowering=False, debug=False, num_devices=NCORES
    )
    pb = nc.dram_tensor(
        "pb", [NPBCH * V, PBW], mybir.dt.bfloat16, kind="ExternalInput"
    ).ap()
    cst = nc.dram_tensor(
        "cst", [128, CW], mybir.dt.bfloat16, kind="ExternalInput"
    ).ap()
    b1c = nc.dram_tensor(
        "b1c", [D, 1], mybir.dt.float32, kind="ExternalInput"
    ).ap()
    b2row = nc.dram_tensor(
        "b2row", [5, D], mybir.dt.bfloat16, kind="ExternalInput"
    ).ap()
    out = nc.dram_tensor(
        "out", [128, R * S], mybir.dt.bfloat16, kind="ExternalOutput"
    ).ap()

    with tile.TileContext(nc) as tc:
        with ExitStack() as ctx:
            _emit(ctx, nc, tc, pb, cst, b1c, b2row, out)
    nc.finalize()
    return nc


def get_module():
    if "nc" not in _CACHE:
        _CACHE["nc"] = _build_module()
    return _CACHE["nc"]


def _build_consts(W1, b1, W2):
    import ml_dtypes

    c = np.zeros((128, CW), np.float32)
    c[:, C_W1R0 : C_W1R0 + D] = W1[0]
    c[:, C_W1R1 : C_W1R1 + D] = W1[1]
    # id-0 mask: row v=0 of both W1 copies is zero, so hpre[0] ignores the
    # histogram and table[0] = encode(0, 0) regardless of the id-0 count
    c[0, C_W1R0 : C_W1R0 + D] = 0.0
    c[0, C_W1R1 : C_W1R1 + D] = 0.0
    c[:, C_B1 : C_B1 + D] = b1
    c[0:D, C_W2 : C_W2 + D] = W2
    c[0:V, C_ID : C_ID + V] = np.eye(V, dtype=np.float32)
    return c.astype(ml_dtypes.bfloat16)


def _build_pb(src, dst):
    # logical pb[v, r*1024 + side*512 + s] = ids[r, s] - v (bf16 exact),
    # stored chunk-major: chunk i at rows [i*V:(i+1)*V], HBM-contiguous
    import ml_dtypes

    ids = np.stack([src, dst], axis=1).astype(np.float32)  # [R, 2, S]
    v = np.arange(V, dtype=np.float32)
    pbf = (ids.reshape(1, R * 2 * S) - v[:, None]).astype(ml_dtypes.bfloat16)
    out = np.zeros((NPBCH * V, PBW), ml_dtypes.bfloat16)
    for i, (a, b) in enumerate(zip(PB_BOUNDS, PB_BOUNDS[1:])):
        out[i * V : (i + 1) * V, 0 : b - a] = pbf[:, a:b]
    return out


def kernel(**inputs):
    global LAST_RESULT
    import ml_dtypes

    from concourse import bass_utils

    src = np.asarray(inputs["src_neighbor_ids"])
    dst = np.asarray(inputs["dst_neighbor_ids"])
    W1 = np.asarray(inputs["W1"], np.float32)
    b1 = np.asarray(inputs["b1"], np.float32)
    W2 = np.asarray(inputs["W2"], np.float32)
    b2 = np.asarray(inputs["b2"], np.float32)

    bf16 = ml_dtypes.bfloat16
    consts = _build_consts(W1, b1, W2)
    b2r = np.tile(b2.reshape(1, D), (5, 1)).astype(bf16)

    in_maps = []
    for c in range(NCORES):
        sl = slice(c * R, (c + 1) * R)
        in_maps.append(
            {
                "pb": _build_pb(src[sl], dst[sl]),
                "cst": consts,
                "b1c": b1.reshape(D, 1),
                "b2row": b2r,
            }
        )

    nc = get_module()
    import os

    trace = bool(int(os.environ.get("KERNEL_TRACE", "0")))
    res = bass_utils.run_bass_kernel_spmd(
        nc, in_maps, core_ids=list(range(NCORES)), trace=trace
    )
    LAST_RESULT = res

    src_feat = np.empty((B, S, D), np.float32)
    dst_feat = np.empty((B, S, D), np.float32)
    for c in range(NCORES):
        o = res.results[c]["out"].astype(np.float32).reshape(128, R, S)
        sl = slice(c * R, (c + 1) * R)
        src_feat[sl] = o[0:D].transpose(1, 2, 0)
        dst_feat[sl] = o[D : 2 * D].transpose(1, 2, 0)
    return src_feat, dst_feat
